# revision 10
# baseline (speedup 1.0000x reference)
"""Multi-head self-attention (B=2, S=2048, D=768, H=12) on 8 trn2 NeuronCores.

Sharding: core c = 4*b + g handles batch b and head-group g (3 heads = 192 of
the 768 model dims). Weights are column-split (wq/wk/wv) and row-split (wo);
each core emits a partial (2048, 768) output; the host sums the 4 group
partials per batch and adds bo.

Device-side dataflow is transpose-free: inputs arrive pre-transposed (D, S),
so projections produce Q^T/K^T in (head_dim, S) layout which feeds the
logits matmul directly; softmax is computed as exp(logits/8) without
max-subtraction (logits are ~N(0,1), exp cannot overflow) with denominators
obtained from a ones-column appended to V in the context matmul; the context
comes out transposed (dims, S), which is exactly the stationary operand the
output projection needs.

Matmul operands use float32r (single-pass ~1.4 cyc/row vs 4+ for fp32, with
~1e-4 matmul precision); accumulation stays fp32 in PSUM.
"""
import numpy as np
from contextlib import ExitStack

import concourse.bacc as bacc
import concourse.mybir as mybir
import concourse.tile as tile
from concourse import bass_utils

# Problem shape (hardcoded per contract).
B, S, D, H, DH = 2, 2048, 768, 12, 64
NCORES = 8
NG = 4            # head groups
HG = H // NG      # heads per group (3)
G = HG * DH       # model dims per group (192)
SC = 512          # query-chunk length
NQ = S // SC      # 4 chunks
KB = 128          # key-block length
NKB = S // KB     # 16 blocks
KT6 = D // 128    # 6 contraction tiles for the projections
SEG = DH + 1      # V segment width per head: 64 V columns + 1 ones column
FP32 = mybir.dt.float32
CDT = mybir.dt.float32r   # matmul-operand dtype

AF = mybir.ActivationFunctionType
ALU = mybir.AluOpType

_CACHE: dict = {}


def _build():
    nc = bacc.Bacc("TRN2", target_bir_lowering=False, debug=False)

    qT = nc.dram_tensor("qT", [D, S], CDT, kind="ExternalInput")
    kT = nc.dram_tensor("kT", [D, S], CDT, kind="ExternalInput")
    vT = nc.dram_tensor("vT", [D, S], CDT, kind="ExternalInput")
    wq = nc.dram_tensor("wq", [D, G], CDT, kind="ExternalInput")
    wk = nc.dram_tensor("wk", [D, G], CDT, kind="ExternalInput")
    wv = nc.dram_tensor("wv", [D, G], CDT, kind="ExternalInput")
    wo = nc.dram_tensor("wo", [G, D], CDT, kind="ExternalInput")
    bq = nc.dram_tensor("bq", [G, 1], FP32, kind="ExternalInput")
    bk = nc.dram_tensor("bk", [G, 1], FP32, kind="ExternalInput")
    bv = nc.dram_tensor("bv", [1, G], CDT, kind="ExternalInput")
    cones = nc.dram_tensor("cones", [1, 1], CDT, kind="ExternalInput")
    yp = nc.dram_tensor("yp", [S, D], FP32, kind="ExternalOutput")

    with tile.TileContext(nc) as tc, ExitStack() as ctx:
        const = ctx.enter_context(tc.tile_pool(name="const", bufs=1))
        xin = ctx.enter_context(tc.tile_pool(name="xin", bufs=2))
        qtp = ctx.enter_context(tc.tile_pool(name="qtp", bufs=2))
        ppool = ctx.enter_context(tc.tile_pool(name="ppool", bufs=2))
        ctxp = ctx.enter_context(tc.tile_pool(name="ctxp", bufs=2))
        ypool = ctx.enter_context(tc.tile_pool(name="ypool", bufs=2))
        den = ctx.enter_context(tc.tile_pool(name="den", bufs=3))
        ps_proj = ctx.enter_context(tc.tile_pool(name="ps_proj", bufs=3, space="PSUM"))
        ps_log = ctx.enter_context(tc.tile_pool(name="ps_log", bufs=3, space="PSUM"))
        ps_ctx = ctx.enter_context(tc.tile_pool(name="ps_ctx", bufs=2, space="PSUM"))

        def rearr6(w):
            return w.ap().rearrange("(t p) g -> p t g", p=128)

        # ---- constants / weights ------------------------------------------
        wq_sb = const.tile([128, KT6, G], CDT)
        nc.sync.dma_start(wq_sb[:], rearr6(wq))
        wk_sb = const.tile([128, KT6, G], CDT)
        nc.sync.dma_start(wk_sb[:], rearr6(wk))
        wv_sb = const.tile([128, KT6, G], CDT)
        nc.sync.dma_start(wv_sb[:], rearr6(wv))
        wo_sb0 = const.tile([128, D], CDT)
        nc.sync.dma_start(wo_sb0[:], wo.ap()[0:128, :])
        wo_sb1 = const.tile([64, D], CDT)
        nc.sync.dma_start(wo_sb1[:], wo.ap()[128:G, :])
        bq0 = const.tile([128, 1], FP32)
        nc.sync.dma_start(bq0[:], bq.ap()[0:128, :])
        bq1 = const.tile([64, 1], FP32)
        nc.sync.dma_start(bq1[:], bq.ap()[128:G, :])
        bk0 = const.tile([128, 1], FP32)
        nc.sync.dma_start(bk0[:], bk.ap()[0:128, :])
        bk1 = const.tile([64, 1], FP32)
        nc.sync.dma_start(bk1[:], bk.ap()[128:G, :])
        bv_sb = const.tile([1, G], CDT)
        nc.sync.dma_start(bv_sb[:], bv.ap()[:, :])
        ones_row = const.tile([1, 128], CDT)
        nc.sync.dma_start(
            ones_row[:], cones.ap()[:, :, None].to_broadcast((1, 128, 1))
        )

        KT0 = const.tile([128, S], CDT)   # K^T heads 0,1 of the group
        KT1 = const.tile([64, S], CDT)    # K^T head 2
        Vg = const.tile([128, NKB, HG * SEG], CDT)  # V blocks + ones columns
        for h in range(HG):
            nc.sync.dma_start(
                Vg[:, :, h * SEG + DH:h * SEG + DH + 1],
                cones.ap().to_broadcast((128, NKB, 1)),
            )

        mblocks = ((128, 0), (64, 128))  # (rows, row-offset) of the 192 dims

        # ---- phase 1: K^T = wk^T @ kT + bk -------------------------------
        for sc_i in range(NQ):
            kx = xin.tile([128, KT6, SC], CDT, tag="xin")
            nc.sync.dma_start(
                kx[:],
                kT.ap()[:, sc_i * SC:(sc_i + 1) * SC].rearrange(
                    "(t p) s -> p t s", p=128
                ),
            )
            for m, (mp, mo) in enumerate(mblocks):
                ps = ps_proj.tile([128, SC], FP32, tag="pp")
                for t in range(KT6):
                    nc.tensor.matmul(
                        ps[:mp, :], wk_sb[:, t, mo:mo + mp], kx[:, t, :],
                        start=(t == 0), stop=(t == KT6 - 1),
                    )
                dst = KT0 if m == 0 else KT1
                bias = bk0 if m == 0 else bk1
                nc.vector.tensor_scalar_add(
                    dst[:mp, sc_i * SC:(sc_i + 1) * SC], ps[:mp, :], bias[:mp, :]
                )

        # ---- phase 2: V = vT^T @ wv + bv (natural layout, + ones cols) ---
        for sb in range(NKB):
            vx = xin.tile([128, KT6, KB], CDT, tag="xin")
            nc.sync.dma_start(
                vx[:],
                vT.ap()[:, sb * KB:(sb + 1) * KB].rearrange(
                    "(t p) s -> p t s", p=128
                ),
            )
            ps = ps_proj.tile([128, G], FP32, tag="pp")
            for t in range(KT6):
                nc.tensor.matmul(
                    ps[:], vx[:, t, :], wv_sb[:, t, :],
                    start=(t == 0), stop=False,
                )
            nc.tensor.matmul(ps[:], ones_row[:], bv_sb[:], start=False, stop=True)
            for h in range(HG):
                nc.vector.tensor_copy(
                    Vg[:, sb, h * SEG:h * SEG + DH], ps[:, h * DH:(h + 1) * DH]
                )

        # ---- phase 3: per q-chunk attention + output projection ----------
        for qc in range(NQ):
            qx = xin.tile([128, KT6, SC], CDT, tag="xin")
            nc.sync.dma_start(
                qx[:],
                qT.ap()[:, qc * SC:(qc + 1) * SC].rearrange(
                    "(t p) s -> p t s", p=128
                ),
            )
            QT0 = qtp.tile([128, SC], CDT, tag="qt0")
            QT1 = qtp.tile([64, SC], CDT, tag="qt1")
            for m, (mp, mo) in enumerate(mblocks):
                ps = ps_proj.tile([128, SC], FP32, tag="pp")
                for t in range(KT6):
                    nc.tensor.matmul(
                        ps[:mp, :], wq_sb[:, t, mo:mo + mp], qx[:, t, :],
                        start=(t == 0), stop=(t == KT6 - 1),
                    )
                dst = QT0 if m == 0 else QT1
                bias = bq0 if m == 0 else bq1
                nc.vector.tensor_scalar_add(dst[:mp, :], ps[:mp, :], bias[:mp, :])

            ctxT0 = ctxp.tile([128, SC], CDT, tag="c0")
            ctxT1 = ctxp.tile([64, SC], CDT, tag="c1")
            for h in range(HG):
                P = ppool.tile([128, NKB, SC], CDT, tag="P")
                if h < 2:
                    kt_t, koff = KT0, 64 * h
                    qt_t, qoff = QT0, 64 * h
                else:
                    kt_t, koff = KT1, 0
                    qt_t, qoff = QT1, 0
                # logits^T (key-major) then P = exp(logits/8)
                for kb in range(NKB):
                    pl = ps_log.tile([128, SC], FP32, tag="pl")
                    nc.tensor.matmul(
                        pl[:],
                        kt_t[koff:koff + DH, kb * KB:(kb + 1) * KB],
                        qt_t[qoff:qoff + DH, :],
                        start=True, stop=True,
                    )
                    nc.scalar.activation(
                        P[:, kb, :], pl[:], AF.Exp, scale=1.0 / np.sqrt(DH)
                    )
                # ctx^T (+ denominator row) = [V | 1]^T @ P
                pc = ps_ctx.tile([SEG, SC], FP32, tag="pc")
                for kb in range(NKB):
                    nc.tensor.matmul(
                        pc[:],
                        Vg[:, kb, h * SEG:(h + 1) * SEG],
                        P[:, kb, :],
                        start=(kb == 0), stop=(kb == NKB - 1),
                    )
                # normalization: recip of the denominator row, broadcast to
                # 64 partitions. The (1, SC) row is spread over 64 partitions
                # first so the iterative-divide reciprocal runs 64-wide.
                cu = den.tile([SEG, SC], FP32, tag="cu")
                nc.vector.tensor_copy(cu[:], pc[:])
                d8 = den.tile([64, SC // 64], FP32, tag="d8")
                nc.sync.dma_start(
                    d8[:],
                    cu[DH:SEG, :].rearrange("o (p f) -> o p f", p=64),
                )
                r8 = den.tile([64, SC // 64], FP32, tag="r8")
                nc.vector.reciprocal(r8[:], d8[:])
                rrow = den.tile([1, SC], FP32, tag="rrow")
                nc.sync.dma_start(
                    rrow[:].rearrange("o (p f) -> o p f", p=64), r8[:]
                )
                rbc = den.tile([64, SC], FP32, tag="rbc")
                nc.sync.dma_start(
                    rbc[:], rrow[:, None, :].to_broadcast((1, 64, SC))
                )
                if h < 2:
                    cdst = ctxT0[64 * h:64 * h + 64, :]
                else:
                    cdst = ctxT1[0:64, :]
                nc.vector.tensor_tensor(cdst, cu[0:DH, :], rbc[:], ALU.mult)

            # output projection: yp_chunk = ctxT^T @ wo
            for half in range(2):
                Yt = ypool.tile([128, 2, D], FP32, tag="Y")
                for m in range(2):
                    sb = half * 2 + m
                    for nh in range(2):
                        py = ps_proj.tile([128, D // 2], FP32, tag="pp")
                        nc.tensor.matmul(
                            py[:],
                            ctxT0[:, sb * 128:(sb + 1) * 128],
                            wo_sb0[:, nh * (D // 2):(nh + 1) * (D // 2)],
                            start=True, stop=False,
                        )
                        nc.tensor.matmul(
                            py[:],
                            ctxT1[:, sb * 128:(sb + 1) * 128],
                            wo_sb1[:, nh * (D // 2):(nh + 1) * (D // 2)],
                            start=False, stop=True,
                        )
                        nc.vector.tensor_copy(
                            Yt[:, m, nh * (D // 2):(nh + 1) * (D // 2)], py[:]
                        )
                nc.sync.dma_start(
                    yp.ap()[
                        qc * SC + half * 256:qc * SC + (half + 1) * 256, :
                    ].rearrange("(m p) d -> p m d", p=128),
                    Yt[:],
                )

    nc.compile()
    return nc


def _get_nc():
    if "nc" not in _CACHE:
        _CACHE["nc"] = _build()
    return _CACHE["nc"]


def _in_maps(v, k, q, wq, bq, wk, bk, wv, bv, wo, bo):
    f32 = lambda a: np.ascontiguousarray(np.asarray(a, dtype=np.float32))
    qTb = [f32(q[b].T) for b in range(B)]
    kTb = [f32(k[b].T) for b in range(B)]
    vTb = [f32(v[b].T) for b in range(B)]
    maps = []
    for c in range(NCORES):
        b, g = divmod(c, NG)
        cols = slice(g * G, (g + 1) * G)
        maps.append({
            "qT": qTb[b],
            "kT": kTb[b],
            "vT": vTb[b],
            "wq": f32(wq[:, cols]),
            "wk": f32(wk[:, cols]),
            "wv": f32(wv[:, cols]),
            "wo": f32(wo[cols, :]),
            "bq": f32(np.asarray(bq)[cols].reshape(G, 1)),
            "bk": f32(np.asarray(bk)[cols].reshape(G, 1)),
            "bv": f32(np.asarray(bv)[cols].reshape(1, G)),
            "cones": np.ones((1, 1), dtype=np.float32),
        })
    return maps


def kernel(v, k, q, wq, bq, wk, bk, wv, bv, wo, bo, _trace=False):
    nc = _get_nc()
    in_maps = _in_maps(v, k, q, wq, bq, wk, bk, wv, bv, wo, bo)
    res = bass_utils.run_bass_kernel_spmd(
        nc, in_maps, core_ids=list(range(NCORES)), trace=_trace
    )
    bo = np.asarray(bo, dtype=np.float32)
    out = np.empty((B, S, D), dtype=np.float32)
    for b in range(B):
        acc = res.results[4 * b]["yp"].astype(np.float32)
        for g in range(1, NG):
            acc = acc + res.results[4 * b + g]["yp"]
        out[b] = acc + bo[None, :]
    if _trace:
        kernel.last_result = res
    return out


# revision 11
# speedup vs baseline: 1.0184x; 1.0184x over previous
"""Multi-head self-attention (B=2, S=2048, D=768, H=12) on 8 trn2 NeuronCores.

Sharding: core c = 4*b + g handles batch b and head-group g (3 heads = 192 of
the 768 model dims). Weights are column-split (wq/wk/wv) and row-split (wo);
each core emits a partial (2048, 768) output; the host sums the 4 group
partials per batch and adds bo.

Device-side dataflow is transpose-free: inputs arrive pre-transposed (D, S),
so projections produce Q^T/K^T in (head_dim, S) layout which feeds the
logits matmul directly; softmax is computed as exp(logits/8) without
max-subtraction (logits are ~N(0,1), exp cannot overflow) with denominators
obtained from a ones-column appended to V in the context matmul; the context
comes out transposed (dims, S), which is exactly the stationary operand the
output projection needs.

Matmul operands use float32r (single-pass ~1.4 cyc/row vs 4+ for fp32, with
~1e-4 matmul precision); accumulation stays fp32 in PSUM.
"""
import numpy as np
from contextlib import ExitStack

import concourse.bacc as bacc
import concourse.mybir as mybir
import concourse.tile as tile
from concourse import bass_utils

# Problem shape (hardcoded per contract).
B, S, D, H, DH = 2, 2048, 768, 12, 64
NCORES = 8
NG = 4            # head groups
HG = H // NG      # heads per group (3)
G = HG * DH       # model dims per group (192)
SC = 512          # query-chunk length
NQ = S // SC      # 4 chunks
KB = 128          # key-block length
NKB = S // KB     # 16 blocks
KT6 = D // 128    # 6 contraction tiles for the projections
SEG = DH + 1      # V segment width per head: 64 V columns + 1 ones column
FP32 = mybir.dt.float32
CDT = mybir.dt.float32r   # matmul-operand dtype

AF = mybir.ActivationFunctionType
ALU = mybir.AluOpType

_CACHE: dict = {}


def _build():
    nc = bacc.Bacc("TRN2", target_bir_lowering=False, debug=False)

    qT = nc.dram_tensor("qT", [NQ, 128, KT6, SC], CDT, kind="ExternalInput")
    kT = nc.dram_tensor("kT", [NQ, 128, KT6, SC], CDT, kind="ExternalInput")
    vT = nc.dram_tensor("vT", [NKB, 128, KT6, KB], CDT, kind="ExternalInput")
    wq = nc.dram_tensor("wq", [128, KT6, G], CDT, kind="ExternalInput")
    wk = nc.dram_tensor("wk", [128, KT6, G], CDT, kind="ExternalInput")
    wv = nc.dram_tensor("wv", [128, KT6, G], CDT, kind="ExternalInput")
    wo = nc.dram_tensor("wo", [G, D], CDT, kind="ExternalInput")
    bq = nc.dram_tensor("bq", [G, 1], FP32, kind="ExternalInput")
    bk = nc.dram_tensor("bk", [G, 1], FP32, kind="ExternalInput")
    bv = nc.dram_tensor("bv", [1, G], CDT, kind="ExternalInput")
    cones = nc.dram_tensor("cones", [1, 1], CDT, kind="ExternalInput")
    yp = nc.dram_tensor("yp", [S, D], FP32, kind="ExternalOutput")

    with tile.TileContext(nc) as tc, ExitStack() as ctx:
        const = ctx.enter_context(tc.tile_pool(name="const", bufs=1))
        xin = ctx.enter_context(tc.tile_pool(name="xin", bufs=2))
        qtp = ctx.enter_context(tc.tile_pool(name="qtp", bufs=2))
        ppool = ctx.enter_context(tc.tile_pool(name="ppool", bufs=2))
        ctxp = ctx.enter_context(tc.tile_pool(name="ctxp", bufs=2))
        ypool = ctx.enter_context(tc.tile_pool(name="ypool", bufs=2))
        den = ctx.enter_context(tc.tile_pool(name="den", bufs=3))
        ps_proj = ctx.enter_context(tc.tile_pool(name="ps_proj", bufs=3, space="PSUM"))
        ps_log = ctx.enter_context(tc.tile_pool(name="ps_log", bufs=3, space="PSUM"))
        ps_ctx = ctx.enter_context(tc.tile_pool(name="ps_ctx", bufs=2, space="PSUM"))

        # ---- constants / weights ------------------------------------------
        wq_sb = const.tile([128, KT6, G], CDT)
        nc.sync.dma_start(wq_sb[:], wq.ap()[:, :, :])
        wk_sb = const.tile([128, KT6, G], CDT)
        nc.sync.dma_start(wk_sb[:], wk.ap()[:, :, :])
        wv_sb = const.tile([128, KT6, G], CDT)
        nc.sync.dma_start(wv_sb[:], wv.ap()[:, :, :])
        wo_sb0 = const.tile([128, D], CDT)
        nc.sync.dma_start(wo_sb0[:], wo.ap()[0:128, :])
        wo_sb1 = const.tile([64, D], CDT)
        nc.sync.dma_start(wo_sb1[:], wo.ap()[128:G, :])
        bq0 = const.tile([128, 1], FP32)
        nc.sync.dma_start(bq0[:], bq.ap()[0:128, :])
        bq1 = const.tile([64, 1], FP32)
        nc.sync.dma_start(bq1[:], bq.ap()[128:G, :])
        bk0 = const.tile([128, 1], FP32)
        nc.sync.dma_start(bk0[:], bk.ap()[0:128, :])
        bk1 = const.tile([64, 1], FP32)
        nc.sync.dma_start(bk1[:], bk.ap()[128:G, :])
        bv_sb = const.tile([1, G], CDT)
        nc.sync.dma_start(bv_sb[:], bv.ap()[:, :])
        ones_row = const.tile([1, 128], CDT)
        nc.sync.dma_start(
            ones_row[:], cones.ap()[:, :, None].to_broadcast((1, 128, 1))
        )

        KT0 = const.tile([128, S], CDT)   # K^T heads 0,1 of the group
        KT1 = const.tile([64, S], CDT)    # K^T head 2
        Vg = const.tile([128, NKB, HG * SEG], CDT)  # V blocks + ones columns
        for h in range(HG):
            nc.sync.dma_start(
                Vg[:, :, h * SEG + DH:h * SEG + DH + 1],
                cones.ap().to_broadcast((128, NKB, 1)),
            )

        mblocks = ((128, 0), (64, 128))  # (rows, row-offset) of the 192 dims

        # ---- phase 1: K^T = wk^T @ kT + bk -------------------------------
        for sc_i in range(NQ):
            kx = xin.tile([128, KT6, SC], CDT, tag="xin")
            nc.sync.dma_start(kx[:], kT.ap()[sc_i])
            for m, (mp, mo) in enumerate(mblocks):
                ps = ps_proj.tile([128, SC], FP32, tag="pp")
                for t in range(KT6):
                    nc.tensor.matmul(
                        ps[:mp, :], wk_sb[:, t, mo:mo + mp], kx[:, t, :],
                        start=(t == 0), stop=(t == KT6 - 1),
                    )
                dst = KT0 if m == 0 else KT1
                bias = bk0 if m == 0 else bk1
                nc.vector.tensor_scalar_add(
                    dst[:mp, sc_i * SC:(sc_i + 1) * SC], ps[:mp, :], bias[:mp, :]
                )

        # ---- phase 2: V = vT^T @ wv + bv (natural layout, + ones cols) ---
        for sb in range(NKB):
            vx = xin.tile([128, KT6, KB], CDT, tag="xin")
            nc.sync.dma_start(vx[:], vT.ap()[sb])
            ps = ps_proj.tile([128, G], FP32, tag="pp")
            for t in range(KT6):
                nc.tensor.matmul(
                    ps[:], vx[:, t, :], wv_sb[:, t, :],
                    start=(t == 0), stop=False,
                )
            nc.tensor.matmul(ps[:], ones_row[:], bv_sb[:], start=False, stop=True)
            for h in range(HG):
                nc.vector.tensor_copy(
                    Vg[:, sb, h * SEG:h * SEG + DH], ps[:, h * DH:(h + 1) * DH]
                )

        # ---- phase 3: per q-chunk attention + output projection ----------
        for qc in range(NQ):
            qx = xin.tile([128, KT6, SC], CDT, tag="xin")
            nc.sync.dma_start(qx[:], qT.ap()[qc])
            QT0 = qtp.tile([128, SC], CDT, tag="qt0")
            QT1 = qtp.tile([64, SC], CDT, tag="qt1")
            for m, (mp, mo) in enumerate(mblocks):
                ps = ps_proj.tile([128, SC], FP32, tag="pp")
                for t in range(KT6):
                    nc.tensor.matmul(
                        ps[:mp, :], wq_sb[:, t, mo:mo + mp], qx[:, t, :],
                        start=(t == 0), stop=(t == KT6 - 1),
                    )
                dst = QT0 if m == 0 else QT1
                bias = bq0 if m == 0 else bq1
                nc.vector.tensor_scalar_add(dst[:mp, :], ps[:mp, :], bias[:mp, :])

            ctxT0 = ctxp.tile([128, SC], CDT, tag="c0")
            ctxT1 = ctxp.tile([64, SC], CDT, tag="c1")
            for h in range(HG):
                P = ppool.tile([128, NKB, SC], CDT, tag="P")
                if h < 2:
                    kt_t, koff = KT0, 64 * h
                    qt_t, qoff = QT0, 64 * h
                else:
                    kt_t, koff = KT1, 0
                    qt_t, qoff = QT1, 0
                # logits^T (key-major) then P = exp(logits/8)
                for kb in range(NKB):
                    pl = ps_log.tile([128, SC], FP32, tag="pl")
                    nc.tensor.matmul(
                        pl[:],
                        kt_t[koff:koff + DH, kb * KB:(kb + 1) * KB],
                        qt_t[qoff:qoff + DH, :],
                        start=True, stop=True,
                    )
                    nc.scalar.activation(
                        P[:, kb, :], pl[:], AF.Exp, scale=1.0 / np.sqrt(DH)
                    )
                # ctx^T (+ denominator row) = [V | 1]^T @ P
                pc = ps_ctx.tile([SEG, SC], FP32, tag="pc")
                for kb in range(NKB):
                    nc.tensor.matmul(
                        pc[:],
                        Vg[:, kb, h * SEG:(h + 1) * SEG],
                        P[:, kb, :],
                        start=(kb == 0), stop=(kb == NKB - 1),
                    )
                # normalization: recip of the denominator row, broadcast to
                # 64 partitions. The (1, SC) row is spread over 64 partitions
                # first so the iterative-divide reciprocal runs 64-wide.
                cu = den.tile([SEG, SC], FP32, tag="cu")
                nc.vector.tensor_copy(cu[:], pc[:])
                d8 = den.tile([64, SC // 64], FP32, tag="d8")
                nc.gpsimd.dma_start(
                    d8[:],
                    cu[DH:SEG, :].rearrange("o (p f) -> o p f", p=64),
                )
                r8 = den.tile([64, SC // 64], FP32, tag="r8")
                nc.vector.reciprocal(r8[:], d8[:])
                rrow = den.tile([1, SC], FP32, tag="rrow")
                nc.gpsimd.dma_start(
                    rrow[:].rearrange("o (p f) -> o p f", p=64), r8[:]
                )
                rbc = den.tile([64, SC], FP32, tag="rbc")
                nc.gpsimd.dma_start(
                    rbc[:], rrow[:, None, :].to_broadcast((1, 64, SC))
                )
                if h < 2:
                    cdst = ctxT0[64 * h:64 * h + 64, :]
                else:
                    cdst = ctxT1[0:64, :]
                nc.vector.tensor_tensor(cdst, cu[0:DH, :], rbc[:], ALU.mult)

            # output projection: yp_chunk = ctxT^T @ wo
            for half in range(2):
                Yt = ypool.tile([128, 2, D], FP32, tag="Y")
                for m in range(2):
                    sb = half * 2 + m
                    for nh in range(2):
                        py = ps_proj.tile([128, D // 2], FP32, tag="pp")
                        nc.tensor.matmul(
                            py[:],
                            ctxT0[:, sb * 128:(sb + 1) * 128],
                            wo_sb0[:, nh * (D // 2):(nh + 1) * (D // 2)],
                            start=True, stop=False,
                        )
                        nc.tensor.matmul(
                            py[:],
                            ctxT1[:, sb * 128:(sb + 1) * 128],
                            wo_sb1[:, nh * (D // 2):(nh + 1) * (D // 2)],
                            start=False, stop=True,
                        )
                        nc.vector.tensor_copy(
                            Yt[:, m, nh * (D // 2):(nh + 1) * (D // 2)], py[:]
                        )
                nc.sync.dma_start(
                    yp.ap()[
                        qc * SC + half * 256:qc * SC + (half + 1) * 256, :
                    ].rearrange("(m p) d -> p m d", p=128),
                    Yt[:],
                )

    nc.compile()
    return nc


def _get_nc():
    if "nc" not in _CACHE:
        _CACHE["nc"] = _build()
    return _CACHE["nc"]


def _tile_x(xb, chunk):
    # x (S, D) -> x^T tiled (S/chunk, 128, KT6, chunk), contiguous
    xt = np.asarray(xb, dtype=np.float32).T
    return np.ascontiguousarray(
        xt.reshape(KT6, 128, S // chunk, chunk).transpose(2, 1, 0, 3)
    )


def _tile_w(w):
    # (D, G) -> (128, KT6, G) contiguous
    w = np.asarray(w, dtype=np.float32)
    return np.ascontiguousarray(w.reshape(KT6, 128, G).transpose(1, 0, 2))


def _in_maps(v, k, q, wq, bq, wk, bk, wv, bv, wo, bo):
    f32 = lambda a: np.ascontiguousarray(np.asarray(a, dtype=np.float32))
    qTb = [_tile_x(q[b], SC) for b in range(B)]
    kTb = [_tile_x(k[b], SC) for b in range(B)]
    vTb = [_tile_x(v[b], KB) for b in range(B)]
    maps = []
    for c in range(NCORES):
        b, g = divmod(c, NG)
        cols = slice(g * G, (g + 1) * G)
        maps.append({
            "qT": qTb[b],
            "kT": kTb[b],
            "vT": vTb[b],
            "wq": _tile_w(np.asarray(wq)[:, cols]),
            "wk": _tile_w(np.asarray(wk)[:, cols]),
            "wv": _tile_w(np.asarray(wv)[:, cols]),
            "wo": f32(wo[cols, :]),
            "bq": f32(np.asarray(bq)[cols].reshape(G, 1)),
            "bk": f32(np.asarray(bk)[cols].reshape(G, 1)),
            "bv": f32(np.asarray(bv)[cols].reshape(1, G)),
            "cones": np.ones((1, 1), dtype=np.float32),
        })
    return maps


def kernel(v, k, q, wq, bq, wk, bk, wv, bv, wo, bo, _trace=False):
    nc = _get_nc()
    in_maps = _in_maps(v, k, q, wq, bq, wk, bk, wv, bv, wo, bo)
    res = bass_utils.run_bass_kernel_spmd(
        nc, in_maps, core_ids=list(range(NCORES)), trace=_trace
    )
    bo = np.asarray(bo, dtype=np.float32)
    out = np.empty((B, S, D), dtype=np.float32)
    for b in range(B):
        acc = res.results[4 * b]["yp"].astype(np.float32)
        for g in range(1, NG):
            acc = acc + res.results[4 * b + g]["yp"]
        out[b] = acc + bo[None, :]
    if _trace:
        kernel.last_result = res
    return out


# revision 13
# speedup vs baseline: 1.1083x; 1.0883x over previous
"""Multi-head self-attention (B=2, S=2048, D=768, H=12) on 8 trn2 NeuronCores.

Sharding: core c = 4*b + g handles batch b and head-group g (3 heads = 192 of
the 768 model dims). Weights are column-split (wq/wk/wv) and row-split (wo);
each core emits a partial (2048, 768) output; the host sums the 4 group
partials per batch and adds bo.

Device-side dataflow is transpose-free: inputs arrive pre-transposed (D, S),
so projections produce Q^T/K^T in (head_dim, S) layout which feeds the
logits matmul directly; softmax is computed as exp(logits/8) without
max-subtraction (logits are ~N(0,1), exp cannot overflow) with denominators
obtained from a ones-column appended to V in the context matmul; the context
comes out transposed (dims, S), which is exactly the stationary operand the
output projection needs.

Matmul operands use float32r (single-pass ~1.4 cyc/row vs 4+ for fp32, with
~1e-4 matmul precision); accumulation stays fp32 in PSUM.
"""
import numpy as np
from contextlib import ExitStack

import concourse.bacc as bacc
import concourse.mybir as mybir
import concourse.tile as tile
from concourse import bass_utils

# Problem shape (hardcoded per contract).
B, S, D, H, DH = 2, 2048, 768, 12, 64
NCORES = 8
NG = 4            # head groups
HG = H // NG      # heads per group (3)
G = HG * DH       # model dims per group (192)
SC = 512          # query-chunk length
NQ = S // SC      # 4 chunks
KB = 128          # key-block length
NKB = S // KB     # 16 blocks
KT6 = D // 128    # 6 contraction tiles for the projections
SEG = DH + 1      # V segment width per head: 64 V columns + 1 ones column
FP32 = mybir.dt.float32
CDT = mybir.dt.float32r   # matmul-operand dtype

AF = mybir.ActivationFunctionType
ALU = mybir.AluOpType

_CACHE: dict = {}


def _build():
    nc = bacc.Bacc("TRN2", target_bir_lowering=False, debug=False)

    qT = nc.dram_tensor("qT", [NQ, 128, KT6, SC], CDT, kind="ExternalInput")
    kT = nc.dram_tensor("kT", [NQ, 128, KT6, SC], CDT, kind="ExternalInput")
    vT = nc.dram_tensor("vT", [NKB, 128, KT6, KB], CDT, kind="ExternalInput")
    wq = nc.dram_tensor("wq", [128, KT6, G], CDT, kind="ExternalInput")
    wk = nc.dram_tensor("wk", [128, KT6, G], CDT, kind="ExternalInput")
    wv = nc.dram_tensor("wv", [128, KT6, G], CDT, kind="ExternalInput")
    wo = nc.dram_tensor("wo", [G, D], CDT, kind="ExternalInput")
    bq = nc.dram_tensor("bq", [G, 1], FP32, kind="ExternalInput")
    bk = nc.dram_tensor("bk", [G, 1], FP32, kind="ExternalInput")
    bv = nc.dram_tensor("bv", [1, G], CDT, kind="ExternalInput")
    yp = nc.dram_tensor("yp", [S, D], FP32, kind="ExternalOutput")

    with tile.TileContext(nc) as tc, ExitStack() as ctx:
        const = ctx.enter_context(tc.tile_pool(name="const", bufs=1))
        xin = ctx.enter_context(tc.tile_pool(name="xin", bufs=2))
        qtp = ctx.enter_context(tc.tile_pool(name="qtp", bufs=2))
        ppool = ctx.enter_context(tc.tile_pool(name="ppool", bufs=2))
        ctxp = ctx.enter_context(tc.tile_pool(name="ctxp", bufs=2))
        ypool = ctx.enter_context(tc.tile_pool(name="ypool", bufs=2))
        den = ctx.enter_context(tc.tile_pool(name="den", bufs=3))
        ps_proj = ctx.enter_context(tc.tile_pool(name="ps_proj", bufs=4, space="PSUM"))
        ps_log = ctx.enter_context(tc.tile_pool(name="ps_log", bufs=2, space="PSUM"))
        ps_ctx = ctx.enter_context(tc.tile_pool(name="ps_ctx", bufs=2, space="PSUM"))

        # ---- constants / weights ------------------------------------------
        wq_sb = const.tile([128, KT6, G], CDT)
        nc.sync.dma_start(wq_sb[:], wq.ap()[:, :, :])
        wk_sb = const.tile([128, KT6, G], CDT)
        nc.sync.dma_start(wk_sb[:], wk.ap()[:, :, :])
        wv_sb = const.tile([128, KT6, G], CDT)
        nc.sync.dma_start(wv_sb[:], wv.ap()[:, :, :])
        wo_sb0 = const.tile([128, D], CDT)
        nc.sync.dma_start(wo_sb0[:], wo.ap()[0:128, :])
        wo_sb1 = const.tile([64, D], CDT)
        nc.sync.dma_start(wo_sb1[:], wo.ap()[128:G, :])
        bq0 = const.tile([128, 1], FP32)
        nc.sync.dma_start(bq0[:], bq.ap()[0:128, :])
        bq1 = const.tile([64, 1], FP32)
        nc.sync.dma_start(bq1[:], bq.ap()[128:G, :])
        bk0 = const.tile([128, 1], FP32)
        nc.sync.dma_start(bk0[:], bk.ap()[0:128, :])
        bk1 = const.tile([64, 1], FP32)
        nc.sync.dma_start(bk1[:], bk.ap()[128:G, :])
        bv_sb = const.tile([1, G], CDT)
        nc.sync.dma_start(bv_sb[:], bv.ap()[:, :])
        # f32r tiles can't be memset directly; build fp32 ones and CAST-copy.
        ones_f32 = const.tile([128, NKB], FP32)
        nc.vector.memset(ones_f32[:], 1.0)
        ones_row = const.tile([1, 128], CDT)
        nc.vector.tensor_copy(
            ones_row[:], ones_f32[0:1, 0:1].to_broadcast((1, 128))
        )

        KT0 = const.tile([128, S], CDT)   # K^T heads 0,1 of the group
        KT1 = const.tile([64, S], CDT)    # K^T head 2
        Vg = const.tile([128, NKB, HG * SEG], CDT)  # V blocks + ones columns
        for h in range(HG):
            nc.vector.tensor_copy(
                Vg[:, :, h * SEG + DH], ones_f32[:, :]
            )

        mblocks = ((128, 0), (64, 128))  # (rows, row-offset) of the 192 dims

        # ---- phase 1: K^T = wk^T @ kT + bk -------------------------------
        for sc_i in range(NQ):
            kx = xin.tile([128, KT6, SC], CDT, tag="xin")
            nc.sync.dma_start(kx[:], kT.ap()[sc_i])
            for m, (mp, mo) in enumerate(mblocks):
                ps = ps_proj.tile([128, SC], FP32, tag="pp")
                for t in range(KT6):
                    nc.tensor.matmul(
                        ps[:mp, :], wk_sb[:, t, mo:mo + mp], kx[:, t, :],
                        start=(t == 0), stop=(t == KT6 - 1),
                    )
                dst = KT0 if m == 0 else KT1
                bias = bk0 if m == 0 else bk1
                nc.vector.tensor_scalar_add(
                    dst[:mp, sc_i * SC:(sc_i + 1) * SC], ps[:mp, :], bias[:mp, :]
                )

        # ---- phase 2: V = vT^T @ wv + bv (natural layout, + ones cols) ---
        for sb in range(NKB):
            vx = xin.tile([128, KT6, KB], CDT, tag="xin")
            nc.sync.dma_start(vx[:], vT.ap()[sb])
            ps = ps_proj.tile([128, G], FP32, tag="pp")
            for t in range(KT6):
                nc.tensor.matmul(
                    ps[:], vx[:, t, :], wv_sb[:, t, :],
                    start=(t == 0), stop=False,
                )
            nc.tensor.matmul(ps[:], ones_row[:], bv_sb[:], start=False, stop=True)
            for h in range(HG):
                nc.vector.tensor_copy(
                    Vg[:, sb, h * SEG:h * SEG + DH], ps[:, h * DH:(h + 1) * DH]
                )

        # ---- phase 3: per q-chunk attention + output projection ----------
        for qc in range(NQ):
            qx = xin.tile([128, KT6, SC], CDT, tag="xin")
            nc.sync.dma_start(qx[:], qT.ap()[qc])
            QT0 = qtp.tile([128, SC], CDT, tag="qt0")
            QT1 = qtp.tile([64, SC], CDT, tag="qt1")
            for m, (mp, mo) in enumerate(mblocks):
                ps = ps_proj.tile([128, SC], FP32, tag="pp")
                for t in range(KT6):
                    nc.tensor.matmul(
                        ps[:mp, :], wq_sb[:, t, mo:mo + mp], qx[:, t, :],
                        start=(t == 0), stop=(t == KT6 - 1),
                    )
                dst = QT0 if m == 0 else QT1
                bias = bq0 if m == 0 else bq1
                nc.vector.tensor_scalar_add(dst[:mp, :], ps[:mp, :], bias[:mp, :])

            ctxT0 = ctxp.tile([128, SC], CDT, tag="c0")
            ctxT1 = ctxp.tile([64, SC], CDT, tag="c1")
            for h in range(HG):
                P = ppool.tile([128, NKB, SC], CDT, tag="P")
                if h < 2:
                    kt_t, koff = KT0, 64 * h
                    qt_t, qoff = QT0, 64 * h
                else:
                    kt_t, koff = KT1, 0
                    qt_t, qoff = QT1, 0
                # logits^T (key-major) then P = exp(logits/8)
                for kb in range(NKB):
                    pl = ps_log.tile([128, SC], FP32, tag="pl")
                    nc.tensor.matmul(
                        pl[:],
                        kt_t[koff:koff + DH, kb * KB:(kb + 1) * KB],
                        qt_t[qoff:qoff + DH, :],
                        start=True, stop=True,
                    )
                    nc.scalar.activation(
                        P[:, kb, :], pl[:], AF.Exp, scale=1.0 / np.sqrt(DH)
                    )
                # ctx^T (+ denominator row) = [V | 1]^T @ P
                pc = ps_ctx.tile([SEG, SC], FP32, tag="pc")
                for kb in range(NKB):
                    nc.tensor.matmul(
                        pc[:],
                        Vg[:, kb, h * SEG:(h + 1) * SEG],
                        P[:, kb, :],
                        start=(kb == 0), stop=(kb == NKB - 1),
                    )
                # normalization: recip of the denominator row, broadcast to
                # 64 partitions. The (1, SC) row is spread over 64 partitions
                # first so the iterative-divide reciprocal runs 64-wide.
                cu = den.tile([SEG, SC], FP32, tag="cu")
                nc.vector.tensor_copy(cu[:], pc[:])
                d8 = den.tile([64, SC // 64], FP32, tag="d8")
                nc.gpsimd.dma_start(
                    d8[:],
                    cu[DH:SEG, :].rearrange("o (p f) -> o p f", p=64),
                )
                r8 = den.tile([64, SC // 64], FP32, tag="r8")
                nc.vector.reciprocal(r8[:], d8[:])
                rrow = den.tile([1, SC], FP32, tag="rrow")
                nc.gpsimd.dma_start(
                    rrow[:].rearrange("o (p f) -> o p f", p=64), r8[:]
                )
                rbc = den.tile([64, SC], FP32, tag="rbc")
                nc.gpsimd.dma_start(
                    rbc[:], rrow[:, None, :].to_broadcast((1, 64, SC))
                )
                if h < 2:
                    cdst = ctxT0[64 * h:64 * h + 64, :]
                else:
                    cdst = ctxT1[0:64, :]
                nc.vector.tensor_tensor(cdst, cu[0:DH, :], rbc[:], ALU.mult)

            # output projection: yp_chunk = ctxT^T @ wo
            for half in range(2):
                Yt = ypool.tile([128, 2, D], FP32, tag="Y")
                for m in range(2):
                    sb = half * 2 + m
                    for nh in range(2):
                        py = ps_proj.tile([128, D // 2], FP32, tag="pp")
                        nc.tensor.matmul(
                            py[:],
                            ctxT0[:, sb * 128:(sb + 1) * 128],
                            wo_sb0[:, nh * (D // 2):(nh + 1) * (D // 2)],
                            start=True, stop=False,
                        )
                        nc.tensor.matmul(
                            py[:],
                            ctxT1[:, sb * 128:(sb + 1) * 128],
                            wo_sb1[:, nh * (D // 2):(nh + 1) * (D // 2)],
                            start=False, stop=True,
                        )
                        nc.vector.tensor_copy(
                            Yt[:, m, nh * (D // 2):(nh + 1) * (D // 2)], py[:]
                        )
                nc.sync.dma_start(
                    yp.ap()[
                        qc * SC + half * 256:qc * SC + (half + 1) * 256, :
                    ].rearrange("(m p) d -> p m d", p=128),
                    Yt[:],
                )

    nc.compile()
    return nc


def _get_nc():
    if "nc" not in _CACHE:
        _CACHE["nc"] = _build()
    return _CACHE["nc"]


def _tile_x(xb, chunk):
    # x (S, D) -> x^T tiled (S/chunk, 128, KT6, chunk), contiguous
    xt = np.asarray(xb, dtype=np.float32).T
    return np.ascontiguousarray(
        xt.reshape(KT6, 128, S // chunk, chunk).transpose(2, 1, 0, 3)
    )


def _tile_w(w):
    # (D, G) -> (128, KT6, G) contiguous
    w = np.asarray(w, dtype=np.float32)
    return np.ascontiguousarray(w.reshape(KT6, 128, G).transpose(1, 0, 2))


def _in_maps(v, k, q, wq, bq, wk, bk, wv, bv, wo, bo):
    f32 = lambda a: np.ascontiguousarray(np.asarray(a, dtype=np.float32))
    qTb = [_tile_x(q[b], SC) for b in range(B)]
    kTb = [_tile_x(k[b], SC) for b in range(B)]
    vTb = [_tile_x(v[b], KB) for b in range(B)]
    maps = []
    for c in range(NCORES):
        b, g = divmod(c, NG)
        cols = slice(g * G, (g + 1) * G)
        maps.append({
            "qT": qTb[b],
            "kT": kTb[b],
            "vT": vTb[b],
            "wq": _tile_w(np.asarray(wq)[:, cols]),
            "wk": _tile_w(np.asarray(wk)[:, cols]),
            "wv": _tile_w(np.asarray(wv)[:, cols]),
            "wo": f32(wo[cols, :]),
            "bq": f32(np.asarray(bq)[cols].reshape(G, 1)),
            "bk": f32(np.asarray(bk)[cols].reshape(G, 1)),
            "bv": f32(np.asarray(bv)[cols].reshape(1, G)),
        })
    return maps


def kernel(v, k, q, wq, bq, wk, bk, wv, bv, wo, bo, _trace=False):
    nc = _get_nc()
    in_maps = _in_maps(v, k, q, wq, bq, wk, bk, wv, bv, wo, bo)
    res = bass_utils.run_bass_kernel_spmd(
        nc, in_maps, core_ids=list(range(NCORES)), trace=_trace
    )
    bo = np.asarray(bo, dtype=np.float32)
    out = np.empty((B, S, D), dtype=np.float32)
    for b in range(B):
        acc = res.results[4 * b]["yp"].astype(np.float32)
        for g in range(1, NG):
            acc = acc + res.results[4 * b + g]["yp"]
        out[b] = acc + bo[None, :]
    if _trace:
        kernel.last_result = res
    return out


# revision 14
# speedup vs baseline: 1.2801x; 1.1550x over previous
"""Multi-head self-attention (B=2, S=2048, D=768, H=12) on 8 trn2 NeuronCores.

Sharding: core c = 4*b + g handles batch b and head-group g (3 heads = 192 of
the 768 model dims). Weights are column-split (wq/wk/wv) and row-split (wo);
each core emits a partial (2048, 768) output; the host sums the 4 group
partials per batch and adds bo.

Device-side dataflow is transpose-free: inputs arrive pre-transposed (D, S),
so projections produce Q^T/K^T in (head_dim, S) layout which feeds the
logits matmul directly; softmax is computed as exp(logits/8) without
max-subtraction (logits are ~N(0,1), exp cannot overflow) with denominators
obtained from a ones-column appended to V in the context matmul; the context
comes out transposed (dims, S), which is exactly the stationary operand the
output projection needs.

Matmul operands use float32r (single-pass ~1.4 cyc/row vs 4+ for fp32, with
~1e-4 matmul precision); accumulation stays fp32 in PSUM.
"""
import numpy as np
from contextlib import ExitStack

import concourse.bacc as bacc
import concourse.mybir as mybir
import concourse.tile as tile
from concourse import bass_utils

# Problem shape (hardcoded per contract).
B, S, D, H, DH = 2, 2048, 768, 12, 64
NCORES = 8
NG = 4            # head groups
HG = H // NG      # heads per group (3)
G = HG * DH       # model dims per group (192)
SC = 512          # query-chunk length
NQ = S // SC      # 4 chunks
KB = 128          # key-block length
NKB = S // KB     # 16 blocks
KT6 = D // 128    # 6 contraction tiles for the projections
SEG = DH + 1      # V segment width per head: 64 V columns + 1 ones column
FP32 = mybir.dt.float32
CDT = mybir.dt.float32r   # matmul-operand dtype

AF = mybir.ActivationFunctionType
ALU = mybir.AluOpType

_CACHE: dict = {}


def _build():
    nc = bacc.Bacc("TRN2", target_bir_lowering=False, debug=False)

    qT = nc.dram_tensor("qT", [NQ, 128, KT6, SC], CDT, kind="ExternalInput")
    kT = nc.dram_tensor("kT", [NQ, 128, KT6, SC], CDT, kind="ExternalInput")
    vT = nc.dram_tensor("vT", [NKB, 128, KT6, KB], CDT, kind="ExternalInput")
    wq = nc.dram_tensor("wq", [128, KT6, G], CDT, kind="ExternalInput")
    wk = nc.dram_tensor("wk", [128, KT6, G], CDT, kind="ExternalInput")
    wv = nc.dram_tensor("wv", [128, KT6, G], CDT, kind="ExternalInput")
    wo = nc.dram_tensor("wo", [G, D], CDT, kind="ExternalInput")
    bq = nc.dram_tensor("bq", [G, 1], FP32, kind="ExternalInput")
    bk = nc.dram_tensor("bk", [G, 1], FP32, kind="ExternalInput")
    bv = nc.dram_tensor("bv", [1, G], CDT, kind="ExternalInput")
    yp = nc.dram_tensor("yp", [S, D], FP32, kind="ExternalOutput")

    with tile.TileContext(nc) as tc, ExitStack() as ctx:
        const = ctx.enter_context(tc.tile_pool(name="const", bufs=1))
        xin = ctx.enter_context(tc.tile_pool(name="xin", bufs=2))
        qtp = ctx.enter_context(tc.tile_pool(name="qtp", bufs=2))
        ppool = ctx.enter_context(tc.tile_pool(name="ppool", bufs=2))
        ctxp = ctx.enter_context(tc.tile_pool(name="ctxp", bufs=2))
        ypool = ctx.enter_context(tc.tile_pool(name="ypool", bufs=2))
        den = ctx.enter_context(tc.tile_pool(name="den", bufs=3))
        ps_proj = ctx.enter_context(tc.tile_pool(name="ps_proj", bufs=2, space="PSUM"))
        ps_log = ctx.enter_context(tc.tile_pool(name="ps_log", bufs=2, space="PSUM"))
        ps_ctx = ctx.enter_context(tc.tile_pool(name="ps_ctx", bufs=2, space="PSUM"))

        # ---- constants / weights ------------------------------------------
        wq_sb = const.tile([128, KT6, G], CDT)
        nc.sync.dma_start(wq_sb[:], wq.ap()[:, :, :])
        wk_sb = const.tile([128, KT6, G], CDT)
        nc.sync.dma_start(wk_sb[:], wk.ap()[:, :, :])
        wv_sb = const.tile([128, KT6, G], CDT)
        nc.sync.dma_start(wv_sb[:], wv.ap()[:, :, :])
        wo_sb0 = const.tile([128, D], CDT)
        nc.sync.dma_start(wo_sb0[:], wo.ap()[0:128, :])
        wo_sb1 = const.tile([64, D], CDT)
        nc.sync.dma_start(wo_sb1[:], wo.ap()[128:G, :])
        bq0 = const.tile([128, 1], FP32)
        nc.sync.dma_start(bq0[:], bq.ap()[0:128, :])
        bq1 = const.tile([64, 1], FP32)
        nc.sync.dma_start(bq1[:], bq.ap()[128:G, :])
        bk0 = const.tile([128, 1], FP32)
        nc.sync.dma_start(bk0[:], bk.ap()[0:128, :])
        bk1 = const.tile([64, 1], FP32)
        nc.sync.dma_start(bk1[:], bk.ap()[128:G, :])
        bv_sb = const.tile([1, G], CDT)
        nc.sync.dma_start(bv_sb[:], bv.ap()[:, :])
        # f32r tiles can't be memset directly; build fp32 ones and CAST-copy.
        ones_f32 = const.tile([128, NKB], FP32)
        nc.vector.memset(ones_f32[:], 1.0)
        ones_row = const.tile([1, 128], CDT)
        nc.vector.tensor_copy(
            ones_row[:], ones_f32[0:1, 0:1].to_broadcast((1, 128))
        )

        KT0 = const.tile([128, S], CDT)   # K^T heads 0,1 of the group
        KT1 = const.tile([64, S], CDT)    # K^T head 2
        Vg = const.tile([128, NKB, HG * SEG], CDT)  # V blocks + ones columns
        for h in range(HG):
            nc.vector.tensor_copy(
                Vg[:, :, h * SEG + DH], ones_f32[:, :]
            )

        mblocks = ((128, 0), (64, 128))  # (rows, row-offset) of the 192 dims

        # ---- phase 1: K^T = wk^T @ kT + bk -------------------------------
        for sc_i in range(NQ):
            kx = xin.tile([128, KT6, SC], CDT, tag="xin")
            nc.sync.dma_start(kx[:], kT.ap()[sc_i])
            for m, (mp, mo) in enumerate(mblocks):
                ps = ps_proj.tile([128, SC], FP32, tag="pp")
                for t in range(KT6):
                    nc.tensor.matmul(
                        ps[:mp, :], wk_sb[:, t, mo:mo + mp], kx[:, t, :],
                        start=(t == 0), stop=(t == KT6 - 1),
                    )
                dst = KT0 if m == 0 else KT1
                bias = bk0 if m == 0 else bk1
                nc.vector.tensor_scalar_add(
                    dst[:mp, sc_i * SC:(sc_i + 1) * SC], ps[:mp, :], bias[:mp, :]
                )

        # ---- phase 2: V = vT^T @ wv + bv (natural layout, + ones cols) ---
        for sb in range(NKB):
            vx = xin.tile([128, KT6, KB], CDT, tag="xin")
            nc.sync.dma_start(vx[:], vT.ap()[sb])
            ps = ps_proj.tile([128, G], FP32, tag="pp")
            for t in range(KT6):
                nc.tensor.matmul(
                    ps[:], vx[:, t, :], wv_sb[:, t, :],
                    start=(t == 0), stop=False,
                )
            nc.tensor.matmul(ps[:], ones_row[:], bv_sb[:], start=False, stop=True)
            for h in range(HG):
                nc.vector.tensor_copy(
                    Vg[:, sb, h * SEG:h * SEG + DH], ps[:, h * DH:(h + 1) * DH]
                )

        # ---- phase 3: per q-chunk attention + output projection ----------
        for qc in range(NQ):
            qx = xin.tile([128, KT6, SC], CDT, tag="xin")
            nc.sync.dma_start(qx[:], qT.ap()[qc])
            QT0 = qtp.tile([128, SC], CDT, tag="qt0")
            QT1 = qtp.tile([64, SC], CDT, tag="qt1")
            for m, (mp, mo) in enumerate(mblocks):
                ps = ps_proj.tile([128, SC], FP32, tag="pp")
                for t in range(KT6):
                    nc.tensor.matmul(
                        ps[:mp, :], wq_sb[:, t, mo:mo + mp], qx[:, t, :],
                        start=(t == 0), stop=(t == KT6 - 1),
                    )
                dst = QT0 if m == 0 else QT1
                bias = bq0 if m == 0 else bq1
                nc.vector.tensor_scalar_add(dst[:mp, :], ps[:mp, :], bias[:mp, :])

            ctxT0 = ctxp.tile([128, SC], CDT, tag="c0")
            ctxT1 = ctxp.tile([64, SC], CDT, tag="c1")
            for h in range(HG):
                P = ppool.tile([128, NKB, SC], CDT, tag="P")
                if h < 2:
                    kt_t, koff = KT0, 64 * h
                    qt_t, qoff = QT0, 64 * h
                else:
                    kt_t, koff = KT1, 0
                    qt_t, qoff = QT1, 0
                # logits^T (key-major) then P = exp(logits/8).
                # Two key-blocks share one 2-bank psum tile so each ACT
                # instruction covers N=1024 (the 352-cycle fixed cost halves).
                for kb2 in range(NKB // 2):
                    pl = ps_log.tile([128, 2, SC], FP32, tag="pl")
                    for j in range(2):
                        kb = 2 * kb2 + j
                        nc.tensor.matmul(
                            pl[:, j, :],
                            kt_t[koff:koff + DH, kb * KB:(kb + 1) * KB],
                            qt_t[qoff:qoff + DH, :],
                            start=True, stop=True,
                        )
                    nc.scalar.activation(
                        P[:, 2 * kb2:2 * kb2 + 2, :], pl[:],
                        AF.Exp, scale=1.0 / np.sqrt(DH)
                    )
                # ctx^T (+ denominator row) = [V | 1]^T @ P
                pc = ps_ctx.tile([SEG, SC], FP32, tag="pc")
                for kb in range(NKB):
                    nc.tensor.matmul(
                        pc[:],
                        Vg[:, kb, h * SEG:(h + 1) * SEG],
                        P[:, kb, :],
                        start=(kb == 0), stop=(kb == NKB - 1),
                    )
                # normalization: recip of the denominator row, broadcast to
                # 64 partitions. The (1, SC) row is spread over 64 partitions
                # first so the iterative-divide reciprocal runs 64-wide.
                cu = den.tile([SEG, SC], FP32, tag="cu")
                nc.vector.tensor_copy(cu[:], pc[:])
                d8 = den.tile([64, SC // 64], FP32, tag="d8")
                nc.gpsimd.dma_start(
                    d8[:],
                    cu[DH:SEG, :].rearrange("o (p f) -> o p f", p=64),
                )
                r8 = den.tile([64, SC // 64], FP32, tag="r8")
                nc.vector.reciprocal(r8[:], d8[:])
                rrow = den.tile([1, SC], FP32, tag="rrow")
                nc.gpsimd.dma_start(
                    rrow[:].rearrange("o (p f) -> o p f", p=64), r8[:]
                )
                rbc = den.tile([64, SC], FP32, tag="rbc")
                nc.gpsimd.dma_start(
                    rbc[:], rrow[:, None, :].to_broadcast((1, 64, SC))
                )
                if h < 2:
                    cdst = ctxT0[64 * h:64 * h + 64, :]
                else:
                    cdst = ctxT1[0:64, :]
                nc.vector.tensor_tensor(cdst, cu[0:DH, :], rbc[:], ALU.mult)

            # output projection: yp_chunk = ctxT^T @ wo
            for half in range(2):
                Yt = ypool.tile([128, 2, D], FP32, tag="Y")
                for m in range(2):
                    sb = half * 2 + m
                    for nh in range(2):
                        py = ps_proj.tile([128, D // 2], FP32, tag="pp")
                        nc.tensor.matmul(
                            py[:],
                            ctxT0[:, sb * 128:(sb + 1) * 128],
                            wo_sb0[:, nh * (D // 2):(nh + 1) * (D // 2)],
                            start=True, stop=False,
                        )
                        nc.tensor.matmul(
                            py[:],
                            ctxT1[:, sb * 128:(sb + 1) * 128],
                            wo_sb1[:, nh * (D // 2):(nh + 1) * (D // 2)],
                            start=False, stop=True,
                        )
                        nc.vector.tensor_copy(
                            Yt[:, m, nh * (D // 2):(nh + 1) * (D // 2)], py[:]
                        )
                nc.sync.dma_start(
                    yp.ap()[
                        qc * SC + half * 256:qc * SC + (half + 1) * 256, :
                    ].rearrange("(m p) d -> p m d", p=128),
                    Yt[:],
                )

    nc.compile()
    return nc


def _get_nc():
    if "nc" not in _CACHE:
        _CACHE["nc"] = _build()
    return _CACHE["nc"]


def _tile_x(xb, chunk):
    # x (S, D) -> x^T tiled (S/chunk, 128, KT6, chunk), contiguous
    xt = np.asarray(xb, dtype=np.float32).T
    return np.ascontiguousarray(
        xt.reshape(KT6, 128, S // chunk, chunk).transpose(2, 1, 0, 3)
    )


def _tile_w(w):
    # (D, G) -> (128, KT6, G) contiguous
    w = np.asarray(w, dtype=np.float32)
    return np.ascontiguousarray(w.reshape(KT6, 128, G).transpose(1, 0, 2))


def _in_maps(v, k, q, wq, bq, wk, bk, wv, bv, wo, bo):
    f32 = lambda a: np.ascontiguousarray(np.asarray(a, dtype=np.float32))
    qTb = [_tile_x(q[b], SC) for b in range(B)]
    kTb = [_tile_x(k[b], SC) for b in range(B)]
    vTb = [_tile_x(v[b], KB) for b in range(B)]
    maps = []
    for c in range(NCORES):
        b, g = divmod(c, NG)
        cols = slice(g * G, (g + 1) * G)
        maps.append({
            "qT": qTb[b],
            "kT": kTb[b],
            "vT": vTb[b],
            "wq": _tile_w(np.asarray(wq)[:, cols]),
            "wk": _tile_w(np.asarray(wk)[:, cols]),
            "wv": _tile_w(np.asarray(wv)[:, cols]),
            "wo": f32(wo[cols, :]),
            "bq": f32(np.asarray(bq)[cols].reshape(G, 1)),
            "bk": f32(np.asarray(bk)[cols].reshape(G, 1)),
            "bv": f32(np.asarray(bv)[cols].reshape(1, G)),
        })
    return maps


def kernel(v, k, q, wq, bq, wk, bk, wv, bv, wo, bo, _trace=False):
    nc = _get_nc()
    in_maps = _in_maps(v, k, q, wq, bq, wk, bk, wv, bv, wo, bo)
    res = bass_utils.run_bass_kernel_spmd(
        nc, in_maps, core_ids=list(range(NCORES)), trace=_trace
    )
    bo = np.asarray(bo, dtype=np.float32)
    out = np.empty((B, S, D), dtype=np.float32)
    for b in range(B):
        acc = res.results[4 * b]["yp"].astype(np.float32)
        for g in range(1, NG):
            acc = acc + res.results[4 * b + g]["yp"]
        out[b] = acc + bo[None, :]
    if _trace:
        kernel.last_result = res
    return out


# revision 16
# speedup vs baseline: 1.4097x; 1.1012x over previous
"""Multi-head self-attention (B=2, S=2048, D=768, H=12) on 8 trn2 NeuronCores.

Sharding: core c = 4*b + g handles batch b and head-group g (3 heads = 192 of
the 768 model dims). Weights are column-split (wq/wk/wv) and row-split (wo);
each core emits a partial (2048, 768) output; the host sums the 4 group
partials per batch and adds bo.

Device-side dataflow is transpose-free: inputs arrive pre-transposed (D, S),
so projections produce Q^T/K^T in (head_dim, S) layout which feeds the
logits matmul directly; softmax is computed as exp(logits/8) without
max-subtraction (logits are ~N(0,1), exp cannot overflow) with denominators
obtained from a ones-column appended to V in the context matmul; the context
comes out transposed (dims, S), which is exactly the stationary operand the
output projection needs.

Matmul operands use float32r (single-pass ~1.4 cyc/row vs 4+ for fp32, with
~1e-4 matmul precision); accumulation stays fp32 in PSUM.
"""
import numpy as np
from contextlib import ExitStack

import concourse.bacc as bacc
import concourse.mybir as mybir
import concourse.tile as tile
from concourse import bass_utils

# Problem shape (hardcoded per contract).
B, S, D, H, DH = 2, 2048, 768, 12, 64
NCORES = 8
NG = 4            # head groups
HG = H // NG      # heads per group (3)
G = HG * DH       # model dims per group (192)
SC = 512          # query-chunk length
NQ = S // SC      # 4 chunks
KB = 128          # key-block length
NKB = S // KB     # 16 blocks
KT6 = D // 128    # 6 contraction tiles for the projections
SEG = DH + 1      # V segment width per head: 64 V columns + 1 ones column
FP32 = mybir.dt.float32
CDT = mybir.dt.float32r   # matmul-operand dtype

AF = mybir.ActivationFunctionType
ALU = mybir.AluOpType

_CACHE: dict = {}


def _build():
    nc = bacc.Bacc("TRN2", target_bir_lowering=False, debug=False)

    qT = nc.dram_tensor("qT", [NQ, 128, KT6, SC], CDT, kind="ExternalInput")
    kT = nc.dram_tensor("kT", [NQ, 128, KT6, SC], CDT, kind="ExternalInput")
    vT = nc.dram_tensor("vT", [NKB, 128, KT6, KB], CDT, kind="ExternalInput")
    wq = nc.dram_tensor("wq", [128, KT6, G], CDT, kind="ExternalInput")
    wk = nc.dram_tensor("wk", [128, KT6, G], CDT, kind="ExternalInput")
    wv = nc.dram_tensor("wv", [128, KT6, G], CDT, kind="ExternalInput")
    wo = nc.dram_tensor("wo", [G, D], CDT, kind="ExternalInput")
    bq = nc.dram_tensor("bq", [G, 1], FP32, kind="ExternalInput")
    bk = nc.dram_tensor("bk", [G, 1], FP32, kind="ExternalInput")
    bv = nc.dram_tensor("bv", [1, G], CDT, kind="ExternalInput")
    yp = nc.dram_tensor("yp", [S, D], FP32, kind="ExternalOutput")

    with tile.TileContext(nc) as tc, ExitStack() as ctx:
        const = ctx.enter_context(tc.tile_pool(name="const", bufs=1))
        xin = ctx.enter_context(tc.tile_pool(name="xin", bufs=2))
        qtp = ctx.enter_context(tc.tile_pool(name="qtp", bufs=2))
        ppool = ctx.enter_context(tc.tile_pool(name="ppool", bufs=2))
        ctxp = ctx.enter_context(tc.tile_pool(name="ctxp", bufs=2))
        ypool = ctx.enter_context(tc.tile_pool(name="ypool", bufs=2))
        den = ctx.enter_context(tc.tile_pool(name="den", bufs=3))
        ps_proj = ctx.enter_context(tc.tile_pool(name="ps_proj", bufs=2, space="PSUM"))
        ps_log = ctx.enter_context(tc.tile_pool(name="ps_log", bufs=2, space="PSUM"))
        ps_ctx = ctx.enter_context(tc.tile_pool(name="ps_ctx", bufs=2, space="PSUM"))

        # ---- constants / weights ------------------------------------------
        wq_sb = const.tile([128, KT6, G], CDT)
        nc.sync.dma_start(wq_sb[:], wq.ap()[:, :, :])
        wk_sb = const.tile([128, KT6, G], CDT)
        nc.sync.dma_start(wk_sb[:], wk.ap()[:, :, :])
        wv_sb = const.tile([128, KT6, G], CDT)
        nc.sync.dma_start(wv_sb[:], wv.ap()[:, :, :])
        wo_sb0 = const.tile([128, D], CDT)
        nc.sync.dma_start(wo_sb0[:], wo.ap()[0:128, :])
        wo_sb1 = const.tile([64, D], CDT)
        nc.sync.dma_start(wo_sb1[:], wo.ap()[128:G, :])
        bq0 = const.tile([128, 1], FP32)
        nc.sync.dma_start(bq0[:], bq.ap()[0:128, :])
        bq1 = const.tile([64, 1], FP32)
        nc.sync.dma_start(bq1[:], bq.ap()[128:G, :])
        bk0 = const.tile([128, 1], FP32)
        nc.sync.dma_start(bk0[:], bk.ap()[0:128, :])
        bk1 = const.tile([64, 1], FP32)
        nc.sync.dma_start(bk1[:], bk.ap()[128:G, :])
        bv_sb = const.tile([1, G], CDT)
        nc.sync.dma_start(bv_sb[:], bv.ap()[:, :])
        # f32r tiles can't be memset directly; build fp32 ones and CAST-copy.
        ones_f32 = const.tile([128, NKB], FP32)
        nc.vector.memset(ones_f32[:], 1.0)
        ones_row = const.tile([1, 128], CDT)
        nc.vector.tensor_copy(
            ones_row[:], ones_f32[0:1, 0:1].to_broadcast((1, 128))
        )

        KT0 = const.tile([128, S], CDT)   # K^T heads 0,1 of the group
        KT1 = const.tile([64, S], CDT)    # K^T head 2
        Vg = const.tile([128, NKB, HG * SEG], CDT)  # V blocks + ones columns
        for h in range(HG):
            nc.vector.tensor_copy(
                Vg[:, :, h * SEG + DH], ones_f32[:, :]
            )

        mblocks = ((128, 0), (64, 128))  # (rows, row-offset) of the 192 dims

        # ---- phase 1: K^T = wk^T @ kT + bk -------------------------------
        for sc_i in range(NQ):
            kx = xin.tile([128, KT6, SC], CDT, tag="xin")
            nc.sync.dma_start(kx[:], kT.ap()[sc_i])
            for m, (mp, mo) in enumerate(mblocks):
                ps = ps_proj.tile([128, SC], FP32, tag="pp")
                for t in range(KT6):
                    nc.tensor.matmul(
                        ps[:mp, :], wk_sb[:, t, mo:mo + mp], kx[:, t, :],
                        start=(t == 0), stop=(t == KT6 - 1),
                    )
                dst = KT0 if m == 0 else KT1
                bias = bk0 if m == 0 else bk1
                nc.vector.tensor_scalar_add(
                    dst[:mp, sc_i * SC:(sc_i + 1) * SC], ps[:mp, :], bias[:mp, :]
                )

        # ---- phase 2: V = vT^T @ wv + bv (natural layout, + ones cols) ---
        for sb in range(NKB):
            vx = xin.tile([128, KT6, KB], CDT, tag="xin")
            nc.sync.dma_start(vx[:], vT.ap()[sb])
            ps = ps_proj.tile([128, G], FP32, tag="pp")
            for t in range(KT6):
                nc.tensor.matmul(
                    ps[:], vx[:, t, :], wv_sb[:, t, :],
                    start=(t == 0), stop=False,
                )
            nc.tensor.matmul(ps[:], ones_row[:], bv_sb[:], start=False, stop=True)
            for h in range(HG):
                nc.vector.tensor_copy(
                    Vg[:, sb, h * SEG:h * SEG + DH], ps[:, h * DH:(h + 1) * DH]
                )

        # ---- phase 3: software-pipelined head stream ---------------------
        # Heads form one flat stream across chunks. Each slot interleaves
        # head i's logits+exp with head i-1's context matmuls so PE and ACT
        # both stay fed (in-order engines execute in emission order). The
        # normalization chain of head i-1 is emitted at slot end; the output
        # projection of a finished chunk is emitted one slot later, after
        # its normalization latency has been hidden under a full slot.
        QT = {}     # qc -> (QT0, QT1)
        CT = {}     # qc -> (ctxT0, ctxT1)

        def head_slices(qc, h):
            qt0, qt1 = QT[qc]
            if h < 2:
                return KT0, 64 * h, qt0, 64 * h
            return KT1, 0, qt1, 0

        def emit_qt_proj(qc):
            qx = xin.tile([128, KT6, SC], CDT, tag="xin")
            nc.sync.dma_start(qx[:], qT.ap()[qc])
            QT0 = qtp.tile([128, SC], CDT, tag="qt0")
            QT1 = qtp.tile([64, SC], CDT, tag="qt1")
            for m, (mp, mo) in enumerate(mblocks):
                ps = ps_proj.tile([128, SC], FP32, tag="pp")
                for t in range(KT6):
                    nc.tensor.matmul(
                        ps[:mp, :], wq_sb[:, t, mo:mo + mp], qx[:, t, :],
                        start=(t == 0), stop=(t == KT6 - 1),
                    )
                dst = QT0 if m == 0 else QT1
                bias = bq0 if m == 0 else bq1
                nc.vector.tensor_scalar_add(dst[:mp, :], ps[:mp, :], bias[:mp, :])
            QT[qc] = (QT0, QT1)

        def emit_norm(qc, h, pc):
            # recip of the denominator row, broadcast to 64 partitions; the
            # (1, SC) row is spread over 64 partitions first so the
            # iterative-divide reciprocal runs 64-wide.
            cu = den.tile([SEG, SC], FP32, tag="cu")
            nc.vector.tensor_copy(cu[:], pc[:])
            d8 = den.tile([64, SC // 64], FP32, tag="d8")
            nc.gpsimd.dma_start(
                d8[:], cu[DH:SEG, :].rearrange("o (p f) -> o p f", p=64)
            )
            r8 = den.tile([64, SC // 64], FP32, tag="r8")
            nc.vector.reciprocal(r8[:], d8[:])
            rrow = den.tile([1, SC], FP32, tag="rrow")
            nc.gpsimd.dma_start(
                rrow[:].rearrange("o (p f) -> o p f", p=64), r8[:]
            )
            rbc = den.tile([64, SC], FP32, tag="rbc")
            nc.gpsimd.dma_start(
                rbc[:], rrow[:, None, :].to_broadcast((1, 64, SC))
            )
            ctxT0, ctxT1 = CT[qc]
            cdst = ctxT0[64 * h:64 * h + 64, :] if h < 2 else ctxT1[0:64, :]
            nc.vector.tensor_tensor(cdst, cu[0:DH, :], rbc[:], ALU.mult)

        def emit_y(qc):
            ctxT0, ctxT1 = CT[qc]
            for half in range(2):
                Yt = ypool.tile([128, 2, D], FP32, tag="Y")
                for m in range(2):
                    sb = half * 2 + m
                    for nh in range(2):
                        py = ps_proj.tile([128, D // 2], FP32, tag="pp")
                        nc.tensor.matmul(
                            py[:],
                            ctxT0[:, sb * 128:(sb + 1) * 128],
                            wo_sb0[:, nh * (D // 2):(nh + 1) * (D // 2)],
                            start=True, stop=False,
                        )
                        nc.tensor.matmul(
                            py[:],
                            ctxT1[:, sb * 128:(sb + 1) * 128],
                            wo_sb1[:, nh * (D // 2):(nh + 1) * (D // 2)],
                            start=False, stop=True,
                        )
                        nc.vector.tensor_copy(
                            Yt[:, m, nh * (D // 2):(nh + 1) * (D // 2)], py[:]
                        )
                nc.sync.dma_start(
                    yp.ap()[
                        qc * SC + half * 256:qc * SC + (half + 1) * 256, :
                    ].rearrange("(m p) d -> p m d", p=128),
                    Yt[:],
                )

        def emit_ctx_pair(prev, kb2):
            qc_p, h_p, P_p, pc_p = prev
            for j in range(2):
                kb = 2 * kb2 + j
                nc.tensor.matmul(
                    pc_p[:],
                    Vg[:, kb, h_p * SEG:(h_p + 1) * SEG],
                    P_p[:, kb, :],
                    start=(kb == 0), stop=(kb == NKB - 1),
                )

        stream = [(qc, h) for qc in range(NQ) for h in range(HG)]
        prev = None      # (qc, h, P, pc) of the head whose ctx is in flight
        y_ready = []     # chunks whose last-head norm was emitted last slot

        for qc, h in stream:
            if h == 0:
                emit_qt_proj(qc)
                CT[qc] = (
                    ctxp.tile([128, SC], CDT, tag="c0", name=f"ctxT0_{qc}"),
                    ctxp.tile([64, SC], CDT, tag="c1", name=f"ctxT1_{qc}"),
                )
            kt_t, koff, qt_t, qoff = head_slices(qc, h)
            P = ppool.tile([128, NKB, SC], CDT, tag="P")
            for kb2 in range(NKB // 2):
                pl = ps_log.tile([128, 2, SC], FP32, tag="pl")
                for j in range(2):
                    kb = 2 * kb2 + j
                    nc.tensor.matmul(
                        pl[:, j, :],
                        kt_t[koff:koff + DH, kb * KB:(kb + 1) * KB],
                        qt_t[qoff:qoff + DH, :],
                        start=True, stop=True,
                    )
                nc.scalar.activation(
                    P[:, 2 * kb2:2 * kb2 + 2, :], pl[:],
                    AF.Exp, scale=1.0 / np.sqrt(DH)
                )
                if prev is not None:
                    emit_ctx_pair(prev, kb2)
            if prev is not None:
                emit_norm(prev[0], prev[1], prev[3])
                if y_ready:
                    emit_y(y_ready.pop(0))
                if prev[1] == HG - 1:
                    y_ready.append(prev[0])
            pc = ps_ctx.tile([SEG, SC], FP32, tag="pc")
            prev = (qc, h, P, pc)

        # flush: context + norm of the final head, then remaining outputs
        for kb2 in range(NKB // 2):
            emit_ctx_pair(prev, kb2)
        emit_norm(prev[0], prev[1], prev[3])
        if prev[1] == HG - 1:
            y_ready.append(prev[0])
        for qc_y in y_ready:
            emit_y(qc_y)

    nc.compile()
    return nc


def _get_nc():
    if "nc" not in _CACHE:
        _CACHE["nc"] = _build()
    return _CACHE["nc"]


def _tile_x(xb, chunk):
    # x (S, D) -> x^T tiled (S/chunk, 128, KT6, chunk), contiguous
    xt = np.asarray(xb, dtype=np.float32).T
    return np.ascontiguousarray(
        xt.reshape(KT6, 128, S // chunk, chunk).transpose(2, 1, 0, 3)
    )


def _tile_w(w):
    # (D, G) -> (128, KT6, G) contiguous
    w = np.asarray(w, dtype=np.float32)
    return np.ascontiguousarray(w.reshape(KT6, 128, G).transpose(1, 0, 2))


def _in_maps(v, k, q, wq, bq, wk, bk, wv, bv, wo, bo):
    f32 = lambda a: np.ascontiguousarray(np.asarray(a, dtype=np.float32))
    qTb = [_tile_x(q[b], SC) for b in range(B)]
    kTb = [_tile_x(k[b], SC) for b in range(B)]
    vTb = [_tile_x(v[b], KB) for b in range(B)]
    maps = []
    for c in range(NCORES):
        b, g = divmod(c, NG)
        cols = slice(g * G, (g + 1) * G)
        maps.append({
            "qT": qTb[b],
            "kT": kTb[b],
            "vT": vTb[b],
            "wq": _tile_w(np.asarray(wq)[:, cols]),
            "wk": _tile_w(np.asarray(wk)[:, cols]),
            "wv": _tile_w(np.asarray(wv)[:, cols]),
            "wo": f32(wo[cols, :]),
            "bq": f32(np.asarray(bq)[cols].reshape(G, 1)),
            "bk": f32(np.asarray(bk)[cols].reshape(G, 1)),
            "bv": f32(np.asarray(bv)[cols].reshape(1, G)),
        })
    return maps


def kernel(v, k, q, wq, bq, wk, bk, wv, bv, wo, bo, _trace=False):
    nc = _get_nc()
    in_maps = _in_maps(v, k, q, wq, bq, wk, bk, wv, bv, wo, bo)
    res = bass_utils.run_bass_kernel_spmd(
        nc, in_maps, core_ids=list(range(NCORES)), trace=_trace
    )
    bo = np.asarray(bo, dtype=np.float32)
    out = np.empty((B, S, D), dtype=np.float32)
    for b in range(B):
        acc = res.results[4 * b]["yp"].astype(np.float32)
        for g in range(1, NG):
            acc = acc + res.results[4 * b + g]["yp"]
        out[b] = acc + bo[None, :]
    if _trace:
        kernel.last_result = res
    return out


# revision 19
# speedup vs baseline: 1.5908x; 1.1284x over previous
"""Multi-head self-attention (B=2, S=2048, D=768, H=12) on 8 trn2 NeuronCores.

Sharding: core c = 4*b + g handles batch b and head-group g (3 heads = 192 of
the 768 model dims). Weights are column-split (wq/wk/wv) and row-split (wo);
each core emits a partial (2048, 768) output; the host sums the 4 group
partials per batch and adds bo.

Device-side dataflow is transpose-free: inputs arrive pre-transposed (D, S),
so projections produce Q^T/K^T in (head_dim, S) layout which feeds the
logits matmul directly; softmax is computed as exp(logits/8) without
max-subtraction (logits are ~N(0,1), exp cannot overflow) with denominators
obtained from a ones-column appended to V in the context matmul; the context
comes out transposed (dims, S), which is exactly the stationary operand the
output projection needs.

Matmul operands use float32r (single-pass ~1.4 cyc/row vs 4+ for fp32, with
~1e-4 matmul precision); accumulation stays fp32 in PSUM.
"""
import numpy as np
from contextlib import ExitStack

import concourse.bacc as bacc
import concourse.mybir as mybir
import concourse.tile as tile
from concourse import bass_utils

# Problem shape (hardcoded per contract).
B, S, D, H, DH = 2, 2048, 768, 12, 64
NCORES = 8
NG = 4            # head groups
HG = H // NG      # heads per group (3)
G = HG * DH       # model dims per group (192)
SC = 512          # query-chunk length
NQ = S // SC      # 4 chunks
KB = 128          # key-block length
NKB = S // KB     # 16 blocks
KT6 = D // 128    # 6 contraction tiles for the projections
SEG = DH + 1      # V segment width per head: 64 V columns + 1 ones column
FP32 = mybir.dt.float32
CDT = mybir.dt.float32r   # matmul-operand dtype

AF = mybir.ActivationFunctionType
ALU = mybir.AluOpType

_CACHE: dict = {}


def _build():
    nc = bacc.Bacc("TRN2", target_bir_lowering=False, debug=False)

    qT = nc.dram_tensor("qT", [NQ, 128, KT6, SC], CDT, kind="ExternalInput")
    kT = nc.dram_tensor("kT", [NQ, 128, KT6, SC], CDT, kind="ExternalInput")
    vT = nc.dram_tensor("vT", [NKB, 128, KT6, KB], CDT, kind="ExternalInput")
    wq = nc.dram_tensor("wq", [128, KT6, G], CDT, kind="ExternalInput")
    wk = nc.dram_tensor("wk", [128, KT6, G], CDT, kind="ExternalInput")
    wv = nc.dram_tensor("wv", [128, KT6, G], CDT, kind="ExternalInput")
    wo = nc.dram_tensor("wo", [G, D], CDT, kind="ExternalInput")
    bq = nc.dram_tensor("bq", [G, 1], FP32, kind="ExternalInput")
    bk = nc.dram_tensor("bk", [G, 1], FP32, kind="ExternalInput")
    bv = nc.dram_tensor("bv", [1, G], CDT, kind="ExternalInput")
    yp = nc.dram_tensor("yp", [S, D], FP32, kind="ExternalOutput")

    with tile.TileContext(nc) as tc, ExitStack() as ctx:
        const = ctx.enter_context(tc.tile_pool(name="const", bufs=1))
        xin = ctx.enter_context(tc.tile_pool(name="xin", bufs=2))
        qtp = ctx.enter_context(tc.tile_pool(name="qtp", bufs=2))
        ppool = ctx.enter_context(tc.tile_pool(name="ppool", bufs=2))
        ctxp = ctx.enter_context(tc.tile_pool(name="ctxp", bufs=2))
        ypool = ctx.enter_context(tc.tile_pool(name="ypool", bufs=2))
        den = ctx.enter_context(tc.tile_pool(name="den", bufs=3))
        ps_proj = ctx.enter_context(tc.tile_pool(name="ps_proj", bufs=2, space="PSUM"))
        ps_log = ctx.enter_context(tc.tile_pool(name="ps_log", bufs=2, space="PSUM"))
        ps_ctx = ctx.enter_context(tc.tile_pool(name="ps_ctx", bufs=2, space="PSUM"))

        # ---- constants / weights ------------------------------------------
        wq_sb = const.tile([128, KT6, G], CDT)
        nc.sync.dma_start(wq_sb[:], wq.ap()[:, :, :])
        wk_sb = const.tile([128, KT6, G], CDT)
        nc.sync.dma_start(wk_sb[:], wk.ap()[:, :, :])
        wv_sb = const.tile([128, KT6, G], CDT)
        nc.sync.dma_start(wv_sb[:], wv.ap()[:, :, :])
        wo_sb0 = const.tile([128, D], CDT)
        nc.sync.dma_start(wo_sb0[:], wo.ap()[0:128, :])
        wo_sb1 = const.tile([64, D], CDT)
        nc.sync.dma_start(wo_sb1[:], wo.ap()[128:G, :])
        bq0 = const.tile([128, 1], FP32)
        nc.sync.dma_start(bq0[:], bq.ap()[0:128, :])
        bq1 = const.tile([64, 1], FP32)
        nc.sync.dma_start(bq1[:], bq.ap()[128:G, :])
        bk0 = const.tile([128, 1], FP32)
        nc.sync.dma_start(bk0[:], bk.ap()[0:128, :])
        bk1 = const.tile([64, 1], FP32)
        nc.sync.dma_start(bk1[:], bk.ap()[128:G, :])
        bv_sb = const.tile([1, G], CDT)
        nc.sync.dma_start(bv_sb[:], bv.ap()[:, :])
        # f32r tiles can't be memset directly; build fp32 ones and CAST-copy.
        ones_f32 = const.tile([128, NKB], FP32)
        nc.vector.memset(ones_f32[:], 1.0)
        ones_row = const.tile([1, 128], CDT)
        nc.vector.tensor_copy(
            ones_row[:], ones_f32[0:1, 0:1].to_broadcast((1, 128))
        )

        KT0 = const.tile([128, S], CDT)   # K^T heads 0,1 of the group
        KT1 = const.tile([64, S], CDT)    # K^T head 2
        Vg = const.tile([128, NKB, HG * SEG], CDT)  # V blocks + ones columns
        for h in range(HG):
            nc.vector.tensor_copy(
                Vg[:, :, h * SEG + DH], ones_f32[:, :]
            )

        mblocks = ((128, 0), (64, 128))  # (rows, row-offset) of the 192 dims

        # ---- phase 1: K^T = wk^T @ kT + bk -------------------------------
        for sc_i in range(NQ):
            kx = xin.tile([128, KT6, SC], CDT, tag="xin")
            nc.sync.dma_start(kx[:], kT.ap()[sc_i])
            for m, (mp, mo) in enumerate(mblocks):
                ps = ps_proj.tile([128, SC], FP32, tag="pp")
                for t in range(KT6):
                    nc.tensor.matmul(
                        ps[:mp, :], wk_sb[:, t, mo:mo + mp], kx[:, t, :],
                        start=(t == 0), stop=(t == KT6 - 1),
                    )
                dst = KT0 if m == 0 else KT1
                bias = bk0 if m == 0 else bk1
                nc.vector.tensor_scalar_add(
                    dst[:mp, sc_i * SC:(sc_i + 1) * SC], ps[:mp, :], bias[:mp, :]
                )

        # ---- phase 2: V = vT^T @ wv + bv (natural layout, + ones cols) ---
        for sb in range(NKB):
            vx = xin.tile([128, KT6, KB], CDT, tag="xin")
            nc.sync.dma_start(vx[:], vT.ap()[sb])
            ps = ps_proj.tile([128, G], FP32, tag="pp")
            for t in range(KT6):
                nc.tensor.matmul(
                    ps[:], vx[:, t, :], wv_sb[:, t, :],
                    start=(t == 0), stop=False,
                )
            nc.tensor.matmul(ps[:], ones_row[:], bv_sb[:], start=False, stop=True)
            for h in range(HG):
                nc.vector.tensor_copy(
                    Vg[:, sb, h * SEG:h * SEG + DH], ps[:, h * DH:(h + 1) * DH]
                )

        # ---- phase 3: software-pipelined head stream ---------------------
        # Heads form one flat stream across chunks. Each slot interleaves
        # head i's logits+exp with head i-1's context matmuls so PE and ACT
        # both stay fed (in-order engines execute in emission order). The
        # normalization chain of head i-1 is emitted at slot end; the output
        # projection of a finished chunk is emitted one slot later, after
        # its normalization latency has been hidden under a full slot.
        QT = {}     # qc -> (QT0, QT1)
        CT = {}     # qc -> (ctxT0, ctxT1)

        def head_slices(qc, h):
            qt0, qt1 = QT[qc]
            if h < 2:
                return KT0, 64 * h, qt0, 64 * h
            return KT1, 0, qt1, 0

        def qt_units(qc):
            # QT projection broken into emission units (PE filler). The qx
            # DMA and tile allocations happen now; matmuls are emitted as
            # the units are drained inside a kb2 loop.
            qx = xin.tile([128, KT6, SC], CDT, tag="xin", name=f"qx_{qc}")
            nc.sync.dma_start(qx[:], qT.ap()[qc])
            QT0 = qtp.tile([128, SC], CDT, tag="qt0", name=f"QT0_{qc}")
            QT1 = qtp.tile([64, SC], CDT, tag="qt1", name=f"QT1_{qc}")
            QT[qc] = (QT0, QT1)
            units = []
            state = {}

            def mk_mm(m, mp, mo, tpair):
                def emit():
                    if m not in state:
                        state[m] = ps_proj.tile(
                            [128, SC], FP32, tag="pp", name=f"qtps_{qc}_{m}"
                        )
                    ps = state[m]
                    for t in tpair:
                        nc.tensor.matmul(
                            ps[:mp, :], wq_sb[:, t, mo:mo + mp], qx[:, t, :],
                            start=(t == 0), stop=(t == KT6 - 1),
                        )
                return emit

            def mk_evict(m, mp):
                def emit():
                    dst = QT0 if m == 0 else QT1
                    bias = bq0 if m == 0 else bq1
                    nc.vector.tensor_scalar_add(
                        dst[:mp, :], state[m][:mp, :], bias[:mp, :]
                    )
                return emit

            for m, (mp, mo) in enumerate(mblocks):
                for tp in ((0, 1), (2, 3), (4, 5)):
                    units.append(mk_mm(m, mp, mo, tp))
                units.append(mk_evict(m, mp))
            return units

        def emit_qt_proj(qc):
            for u in qt_units(qc):
                u()

        def emit_norm(qc, h, pc):
            # recip of the denominator row, broadcast to 64 partitions; the
            # (1, SC) row is spread over 64 partitions first so the
            # iterative-divide reciprocal runs 64-wide.
            cu = den.tile([SEG, SC], FP32, tag="cu")
            nc.vector.tensor_copy(cu[:], pc[:])
            d8 = den.tile([64, SC // 64], FP32, tag="d8")
            nc.gpsimd.dma_start(
                d8[:], cu[DH:SEG, :].rearrange("o (p f) -> o p f", p=64)
            )
            r8 = den.tile([64, SC // 64], FP32, tag="r8")
            nc.vector.reciprocal(r8[:], d8[:])
            rrow = den.tile([1, SC], FP32, tag="rrow")
            nc.gpsimd.dma_start(
                rrow[:].rearrange("o (p f) -> o p f", p=64), r8[:]
            )
            rbc = den.tile([64, SC], FP32, tag="rbc")
            nc.gpsimd.dma_start(
                rbc[:], rrow[:, None, :].to_broadcast((1, 64, SC))
            )
            ctxT0, ctxT1 = CT[qc]
            cdst = ctxT0[64 * h:64 * h + 64, :] if h < 2 else ctxT1[0:64, :]
            nc.vector.tensor_tensor(cdst, cu[0:DH, :], rbc[:], ALU.mult)

        def y_units(qc):
            # Output projection as emission units (PE filler): 8 units of
            # two accumulating matmuls + psum eviction; the chunk-half DMA
            # rides on its last unit.
            ctxT0, ctxT1 = CT[qc]
            ytiles = {}
            units = []

            def mk_unit(half, m, nh):
                def emit():
                    if half not in ytiles:
                        ytiles[half] = ypool.tile(
                            [128, 2, D], FP32, tag="Y", name=f"Yt_{qc}_{half}"
                        )
                    Yt = ytiles[half]
                    sb = half * 2 + m
                    py = ps_proj.tile(
                        [128, D // 2], FP32, tag="pp", name=f"yps_{qc}_{sb}_{nh}"
                    )
                    nc.tensor.matmul(
                        py[:],
                        ctxT0[:, sb * 128:(sb + 1) * 128],
                        wo_sb0[:, nh * (D // 2):(nh + 1) * (D // 2)],
                        start=True, stop=False,
                    )
                    nc.tensor.matmul(
                        py[:],
                        ctxT1[:, sb * 128:(sb + 1) * 128],
                        wo_sb1[:, nh * (D // 2):(nh + 1) * (D // 2)],
                        start=False, stop=True,
                    )
                    nc.vector.tensor_copy(
                        Yt[:, m, nh * (D // 2):(nh + 1) * (D // 2)], py[:]
                    )
                    if m == 1 and nh == 1:
                        nc.sync.dma_start(
                            yp.ap()[
                                qc * SC + half * 256:
                                qc * SC + (half + 1) * 256, :
                            ].rearrange("(m p) d -> p m d", p=128),
                            Yt[:],
                        )
                return emit

            for half in range(2):
                for m in range(2):
                    for nh in range(2):
                        units.append(mk_unit(half, m, nh))
            return units

        def emit_y(qc):
            for u in y_units(qc):
                u()

        def emit_ctx_pair(prev, kb2):
            qc_p, h_p, P_p, pc_p = prev
            for j in range(2):
                kb = 2 * kb2 + j
                nc.tensor.matmul(
                    pc_p[:],
                    Vg[:, kb, h_p * SEG:(h_p + 1) * SEG],
                    P_p[:, kb, :],
                    start=(kb == 0), stop=(kb == NKB - 1),
                )

        stream = [(qc, h) for qc in range(NQ) for h in range(HG)]
        prev = None      # (qc, h, P, pc) of the head whose ctx is in flight

        # QT(0) must fully precede its own logits, so it is emitted as a
        # block; later chunks' QT projections are prefetched as filler
        # inside the previous chunk's last head slot.
        emit_qt_proj(0)

        for qc, h in stream:
            if h == 0:
                CT[qc] = (
                    ctxp.tile([128, SC], CDT, tag="c0", name=f"ctxT0_{qc}"),
                    ctxp.tile([64, SC], CDT, tag="c1", name=f"ctxT1_{qc}"),
                )
            # PE filler for this slot: Y of the chunk finished last slot
            # (h==1: its normalization has had a full slot to land), or the
            # next chunk's QT projection prefetch (h==2).
            filler = []
            start_iter = 0
            if h == 1 and qc >= 1:
                filler = y_units(qc - 1)
                start_iter = 3
            elif h == HG - 1 and qc + 1 < NQ:
                filler = qt_units(qc + 1)

            kt_t, koff, qt_t, qoff = head_slices(qc, h)
            P = ppool.tile([128, NKB, SC], CDT, tag="P")
            NIT = NKB // 2
            for kb2 in range(NIT):
                pl = ps_log.tile([128, 2, SC], FP32, tag="pl")
                for j in range(2):
                    kb = 2 * kb2 + j
                    nc.tensor.matmul(
                        pl[:, j, :],
                        kt_t[koff:koff + DH, kb * KB:(kb + 1) * KB],
                        qt_t[qoff:qoff + DH, :],
                        start=True, stop=True,
                    )
                nc.scalar.activation(
                    P[:, 2 * kb2:2 * kb2 + 2, :], pl[:],
                    AF.Exp, scale=1.0 / np.sqrt(DH)
                )
                if prev is not None:
                    emit_ctx_pair(prev, kb2)
                if filler and kb2 >= start_iter:
                    n = -(-len(filler) // (NIT - kb2))
                    for _ in range(n):
                        filler.pop(0)()
            for u in filler:
                u()
            if prev is not None:
                emit_norm(prev[0], prev[1], prev[3])
            pc = ps_ctx.tile([SEG, SC], FP32, tag="pc")
            prev = (qc, h, P, pc)

        # flush: context + norm of the final head, then its chunk's output
        for kb2 in range(NKB // 2):
            emit_ctx_pair(prev, kb2)
        emit_norm(prev[0], prev[1], prev[3])
        emit_y(NQ - 1)

    nc.compile()
    return nc


def _get_nc():
    if "nc" not in _CACHE:
        _CACHE["nc"] = _build()
    return _CACHE["nc"]


def _tile_x(xb, chunk):
    # x (S, D) -> x^T tiled (S/chunk, 128, KT6, chunk), contiguous
    xt = np.asarray(xb, dtype=np.float32).T
    return np.ascontiguousarray(
        xt.reshape(KT6, 128, S // chunk, chunk).transpose(2, 1, 0, 3)
    )


def _tile_w(w):
    # (D, G) -> (128, KT6, G) contiguous
    w = np.asarray(w, dtype=np.float32)
    return np.ascontiguousarray(w.reshape(KT6, 128, G).transpose(1, 0, 2))


def _in_maps(v, k, q, wq, bq, wk, bk, wv, bv, wo, bo):
    f32 = lambda a: np.ascontiguousarray(np.asarray(a, dtype=np.float32))
    qTb = [_tile_x(q[b], SC) for b in range(B)]
    kTb = [_tile_x(k[b], SC) for b in range(B)]
    vTb = [_tile_x(v[b], KB) for b in range(B)]
    maps = []
    for c in range(NCORES):
        b, g = divmod(c, NG)
        cols = slice(g * G, (g + 1) * G)
        maps.append({
            "qT": qTb[b],
            "kT": kTb[b],
            "vT": vTb[b],
            "wq": _tile_w(np.asarray(wq)[:, cols]),
            "wk": _tile_w(np.asarray(wk)[:, cols]),
            "wv": _tile_w(np.asarray(wv)[:, cols]),
            "wo": f32(wo[cols, :]),
            "bq": f32(np.asarray(bq)[cols].reshape(G, 1)),
            "bk": f32(np.asarray(bk)[cols].reshape(G, 1)),
            "bv": f32(np.asarray(bv)[cols].reshape(1, G)),
        })
    return maps


def kernel(v, k, q, wq, bq, wk, bk, wv, bv, wo, bo, _trace=False):
    nc = _get_nc()
    in_maps = _in_maps(v, k, q, wq, bq, wk, bk, wv, bv, wo, bo)
    res = bass_utils.run_bass_kernel_spmd(
        nc, in_maps, core_ids=list(range(NCORES)), trace=_trace
    )
    bo = np.asarray(bo, dtype=np.float32)
    out = np.empty((B, S, D), dtype=np.float32)
    for b in range(B):
        acc = res.results[4 * b]["yp"].astype(np.float32)
        for g in range(1, NG):
            acc = acc + res.results[4 * b + g]["yp"]
        out[b] = acc + bo[None, :]
    if _trace:
        kernel.last_result = res
    return out
